# revision 1
# baseline (speedup 1.0000x reference)
"""GQA attention (B=4, S=1024, H=4096, 32 q heads / 8 kv heads, rotary) on 8 trn2 cores.

Sharding: DP4 x TP2. Core c = 2*b + j handles batch b with kv-head half j:
  - column-parallel wq/wk/wv (16 q heads / 4 kv heads per core)
  - row-parallel wo -> partial [S, H] outputs, host sums core pairs.

Per-core dataflow (matmuls in fp32r = TF32 @ 1 cyc/row, transpose-free):
  qT = wq.T @ xT   [2048, 1024]  (channels on partitions -> scores lhsT/rhs layout)
  kT = wk.T @ xT   [512, 1024]
  v  = xT.T @ wv   [1024, 512]   (natural layout -> attn@v lhsT) via PE transpose of vT
  rope on qT/kT with host-precomputed sin/cos maps; channel pairs are
  host-permuted into partition halves so the pair-mix is a partition-window op.
  scoresT[t,s] = kT.T @ qT ; exp on ACT (no max-sub needed, scores are small);
  denom = ones.T @ expT (PE column-sum); bcast 1/denom via K=1 matmul;
  oT[d,s] = v.T @ expT * inv ; out = oT.T @ wo.
"""

import numpy as np

B = 4
S = 1024
H = 4096
D = 128
HQ = 32
HKV = 8
G = 4
NCORES = 8
QC = 2048  # q cols per core
KC = 512  # k cols per core
VC = 512  # v cols per core
COH = 2048  # wo rows per core
ROPE_BASE = 10000.0

ROPE_DVE_OFFSET = False  # partition-offset DVE inputs are illegal on HW (NCC_IBIR297)

_CACHE = {}


def _build(reps=1):
    import concourse.tile as tile
    from concourse import bacc, mybir
    from concourse.masks import make_identity

    fp32 = mybir.dt.float32
    f32r = mybir.dt.float32r

    nc = bacc.Bacc(None, target_bir_lowering=False)

    xT_d = nc.dram_tensor("xT", [H, S], f32r, kind="ExternalInput")
    wq_d = nc.dram_tensor("wq", [H, QC], f32r, kind="ExternalInput")
    wk_d = nc.dram_tensor("wk", [H, KC], f32r, kind="ExternalInput")
    wv_d = nc.dram_tensor("wv", [H, VC], f32r, kind="ExternalInput")
    wo_d = nc.dram_tensor("wo", [COH, H], f32r, kind="ExternalInput")
    aq_d = nc.dram_tensor("ropeAq", [D, S], fp32, kind="ExternalInput")
    bq_d = nc.dram_tensor("ropeBq", [D, S], fp32, kind="ExternalInput")
    ak_d = nc.dram_tensor("ropeAk", [D, S], fp32, kind="ExternalInput")
    bk_d = nc.dram_tensor("ropeBk", [D, S], fp32, kind="ExternalInput")
    out_d = nc.dram_tensor("out", [S, H], fp32, kind="ExternalOutput")
    qspill_d = nc.dram_tensor("qspill", [16, D, S], f32r)  # internal scratch
    stash_d = nc.dram_tensor("stash", [24, D, S], fp32)  # k-half partial sums

    xT_r = xT_d.rearrange("(ko p) t -> p ko t", p=128)  # [128, 32, 1024]
    wq_r = wq_d.rearrange("(ko p) c -> p ko c", p=128)
    wk_r = wk_d.rearrange("(ko p) c -> p ko c", p=128)
    wv_r = wv_d.rearrange("(ko p) c -> p ko c", p=128)
    wo_r = wo_d.rearrange("(co p) h -> p co h", p=128)  # [128, 16, 4096]
    out_r = out_d.rearrange("(tb p) h -> tb p h", p=128)  # [8, 128, 4096]

    NKO = H // 128  # 32 contraction tiles
    KOC = 4  # ko tiles per xt chunk
    NCHUNK = NKO // KOC  # 4 chunks per t-half

    with tile.TileContext(nc) as tc, nc.allow_low_precision(
        reason="fp32r (tf32) matmul pipeline"
    ):
      for _rep in range(reps):
        with (
            tc.tile_pool(name="persist", bufs=1) as persist,
            tc.tile_pool(name="konst", bufs=1) as konst,
            tc.tile_pool(name="qt", bufs=4) as qpool,
        ):
            kT = persist.tile([128, HKV // 2, S], f32r)  # [128, 4, 1024]
            v = persist.tile([128, S // 128, VC], f32r)  # [128, 8, 512]
            ones_f = konst.tile([128, 128], fp32)
            nc.vector.memset(ones_f[:], 1.0)
            ones = konst.tile([128, 128], f32r)
            nc.vector.tensor_copy(ones[:], ones_f[:])
            ident = konst.tile([128, 128], fp32)
            make_identity(nc, ident[:])

            # ---------------- Phase 1: projections + rope ----------------
            with (
                tc.tile_pool(name="maps", bufs=1) as mpool,
                tc.tile_pool(name="xt", bufs=NCHUNK + 3) as xpool,
                tc.tile_pool(name="wt", bufs=2) as wpool,
                tc.tile_pool(name="ev", bufs=2) as epool,
                tc.tile_pool(name="stash", bufs=4) as spool,
                tc.tile_pool(name="ps1", bufs=6, space="PSUM") as ps1,
                tc.tile_pool(name="pst", bufs=2, space="PSUM") as pst,
            ):
                maps = {}

                NKO2 = NKO // 2  # 16 ko per K-half
                KOC2 = 2  # ko tiles per xt chunk (full-t chunks)
                NCH2 = NKO2 // KOC2  # 8 chunks per K-half

                sched = (
                    [(wq_r, cb, "q") for cb in range(16)]
                    + [(wk_r, cb, "k") for cb in range(4)]
                    + [(wv_r, cb, "v") for cb in range(4)]
                )

                def load_wt(w_r, cb, kh):
                    wt = wpool.tile([128, NKO2, 128], f32r, tag="wt", name="wt")
                    nc.sync.dma_start(
                        wt[:],
                        w_r[:, kh * NKO2 : (kh + 1) * NKO2,
                            cb * 128 : (cb + 1) * 128],
                    )
                    return wt

                def rope_evict(raw, Am, Bm, out_ap, th):
                    ts_ = slice(th * 512, th * 512 + 512)
                    t1 = epool.tile([128, 512], fp32, tag="t1", name="t1")
                    nc.vector.tensor_mul(t1[:], raw[:], Am[:, ts_])
                    sw = epool.tile([128, 512], fp32, tag="sw", name="sw")
                    nc.sync.dma_start(sw[0:64, :], raw[64:128, :])
                    nc.sync.dma_start(sw[64:128, :], raw[0:64, :])
                    t2 = epool.tile([128, 512], fp32, tag="t2", name="t2")
                    nc.vector.tensor_mul(t2[:], sw[:], Bm[:, ts_])
                    nc.vector.tensor_add(out_ap, t1[:], t2[:])

                wt_next = [None]
                stash_next = []

                def load_stash(i):
                    st = spool.tile([128, S], fp32, tag="stash", name="stash")
                    nc.sync.dma_start(st[:], stash_d[i])
                    return st

                for kh in range(2):

                    def load_chunk(ch):
                        xt = xpool.tile(
                            [128, KOC2, S], f32r, tag="xt", name=f"xt{kh}_{ch}"
                        )
                        base = kh * NKO2 + ch * KOC2
                        nc.sync.dma_start(xt[:], xT_r[:, base : base + KOC2, :])
                        return xt

                    # priority: first chunk, first weight, rest, maps
                    xts = [load_chunk(0)]
                    if wt_next[0] is None:
                        wt_next[0] = load_wt(wq_r, 0, 0)
                    for ch in range(1, NCH2):
                        xts.append(load_chunk(ch))
                    if kh == 1:
                        stash_next.extend([load_stash(0), load_stash(1)])
                        for nm, dram in (
                            ("Aq", aq_d), ("Bq", bq_d), ("Ak", ak_d), ("Bk", bk_d)
                        ):
                            mt = mpool.tile([128, S], fp32, name=nm)
                            nc.sync.dma_start(mt[:], dram[:])
                            maps[nm] = mt
                        Aq, Bq, Ak, Bk = (
                            maps["Aq"], maps["Bq"], maps["Ak"], maps["Bk"]
                        )

                    for i, (w_r, cb, kind) in enumerate(sched):
                        wt = wt_next[0]
                        psA = ps1.tile([128, 512], fp32, tag="ps1", name="psA")
                        psB = ps1.tile([128, 512], fp32, tag="ps1", name="psB")
                        for ko in range(NKO2):
                            xt = xts[ko // KOC2]
                            nc.tensor.matmul(
                                psA[:],
                                wt[:, ko, :],
                                xt[:, ko % KOC2, 0:512],
                                start=(ko == 0),
                                stop=(ko == NKO2 - 1),
                            )
                            nc.tensor.matmul(
                                psB[:],
                                wt[:, ko, :],
                                xt[:, ko % KOC2, 512:1024],
                                start=(ko == 0),
                                stop=(ko == NKO2 - 1),
                            )
                        # prefetch next weight (next sched item, or K-half 1)
                        if i + 1 < len(sched):
                            wt_next[0] = load_wt(sched[i + 1][0], sched[i + 1][1], kh)
                        elif kh == 0:
                            wt_next[0] = load_wt(wq_r, 0, 1)

                        if kh == 0:
                            st = spool.tile([128, S], fp32, tag="stash", name="stout")
                            nc.scalar.copy(st[:, 0:512], psA[:])
                            nc.scalar.copy(st[:, 512:1024], psB[:])
                            nc.sync.dma_start(stash_d[i], st[:])
                            continue

                        # kh == 1: combine with stash, rope/evict
                        st = stash_next.pop(0)
                        if i + 2 < len(sched):
                            stash_next.append(load_stash(i + 2))
                        for th, ps in ((0, psA), (1, psB)):
                            ts_ = slice(th * 512, th * 512 + 512)
                            raw = epool.tile([128, 512], fp32, tag="raw", name="raw")
                            nc.vector.tensor_add(raw[:], st[:, ts_], ps[:])
                            if kind == "q":
                                y = epool.tile([128, 512], f32r, tag="y", name="y")
                                rope_evict(raw, Aq, Bq, y[:], th)
                                nc.sync.dma_start(qspill_d[cb, :, ts_], y[:])
                            elif kind == "k":
                                rope_evict(raw, Ak, Bk, kT[:, cb, ts_], th)
                            else:  # v: PE-transpose into natural v
                                for j in range(4):
                                    pt = pst.tile(
                                        [128, 128], fp32, tag="pst", name="pt"
                                    )
                                    nc.tensor.transpose(
                                        pt[:],
                                        raw[:, j * 128 : (j + 1) * 128],
                                        ident[:],
                                    )
                                    nc.vector.tensor_copy(
                                        v[:, th * 4 + j, cb * 128 : (cb + 1) * 128],
                                        pt[:],
                                    )

            # ---------------- Phase 2+3 ----------------
            with (
                tc.tile_pool(name="ot", bufs=1) as opool,
                tc.tile_pool(name="wopre", bufs=1) as wopre,
            ):
                oT = opool.tile([128, 16, S], f32r)  # 64 KiB/part

                def _wo_dma(wot, hh, half):
                    nc.sync.dma_start(
                        wot[:],
                        wo_r[:, half * 8 : (half + 1) * 8,
                             hh * 512 : (hh + 1) * 512],
                    )
                    return wot

                # prefetch both hh=0 wo strips during attention
                wot00 = wopre.tile([128, 8, 512], f32r, name="wot00")
                wo_next = [_wo_dma(wot00, 0, 0)]
                wot01 = wopre.tile([128, 8, 512], f32r, name="wot01")
                _wo_dma(wot01, 0, 1)

                with (
                    tc.tile_pool(name="ex", bufs=2) as expool,
                    tc.tile_pool(name="sm", bufs=3) as smpool,
                    tc.tile_pool(name="pssc", bufs=4, space="PSUM") as pssc,
                    tc.tile_pool(name="psden", bufs=1, space="PSUM") as psden,
                    tc.tile_pool(name="psbc", bufs=1, space="PSUM") as psbc,
                    tc.tile_pool(name="pso", bufs=2, space="PSUM") as pso,
                ):
                    for h in range(4):
                        for g in range(4):
                            cb = h * 4 + g
                            for sh in range(2):
                                ss = slice(sh * 512, sh * 512 + 512)
                                qt = qpool.tile([128, 512], f32r, tag="qt", name="qt")
                                nc.sync.dma_start(qt[:], qspill_d[cb, :, ss])
                                expT = expool.tile(
                                    [128, 8, 512], f32r, tag="expT", name="expT"
                                )
                                for tb in range(8):
                                    psc = pssc.tile(
                                        [128, 512], fp32, tag="psc", name="psc"
                                    )
                                    nc.tensor.matmul(
                                        psc[:],
                                        kT[:, h, tb * 128 : (tb + 1) * 128],
                                        qt[:],
                                        start=True,
                                        stop=True,
                                    )
                                    nc.scalar.activation(
                                        expT[:, tb],
                                        psc[:],
                                        mybir.ActivationFunctionType.Exp,
                                    )
                                pden = psden.tile([1, 512], fp32, tag="pd", name="pd")
                                for tb in range(8):
                                    nc.tensor.matmul(
                                        pden[:],
                                        ones[:, 0:1],
                                        expT[:, tb],
                                        start=(tb == 0),
                                        stop=(tb == 7),
                                    )
                                inv = smpool.tile([1, 512], f32r, tag="inv", name="inv")
                                nc.vector.reciprocal(inv[:], pden[:])
                                pbc = psbc.tile([128, 512], fp32, tag="pb", name="pb")
                                nc.tensor.matmul(
                                    pbc[:], ones[0:1, :], inv[:],
                                    start=True, stop=True,
                                )
                                invb = smpool.tile(
                                    [128, 512], fp32, tag="invb", name="invb"
                                )
                                nc.vector.tensor_copy(invb[:], pbc[:])
                                po = pso.tile([128, 512], fp32, tag="po", name="po")
                                for tb in range(8):
                                    nc.tensor.matmul(
                                        po[:],
                                        v[:, tb, h * 128 : (h + 1) * 128],
                                        expT[:, tb],
                                        start=(tb == 0),
                                        stop=(tb == 7),
                                    )
                                nc.vector.tensor_mul(oT[:, cb, ss], po[:], invb[:])

                # Phase 3: out = oT.T @ wo
                with (
                    tc.tile_pool(name="wot", bufs=2) as wopool,
                    tc.tile_pool(name="outp", bufs=2) as outpool,
                    tc.tile_pool(name="psout", bufs=3, space="PSUM") as psout,
                ):
                    def load_wo_strip(hh, half):
                        wot = wopool.tile(
                            [128, 8, 512], f32r, tag=f"wo{half}", name=f"wo{half}"
                        )
                        return _wo_dma(wot, hh, half)

                    for hh in range(8):
                        hs = slice(hh * 512, hh * 512 + 512)
                        wotA = wo_next[0]
                        wotB = wot01 if hh == 0 else load_wo_strip(hh, 1)
                        for tb in range(8):
                            pso_ = psout.tile([128, 512], fp32, tag="pso", name="pso_")
                            for co in range(8):
                                nc.tensor.matmul(
                                    pso_[:],
                                    oT[:, co, tb * 128 : (tb + 1) * 128],
                                    wotA[:, co, :],
                                    start=(co == 0),
                                    stop=False,
                                )
                            if tb == 0 and hh < 7:
                                wo_next[0] = load_wo_strip(hh + 1, 0)
                            for co in range(8, 16):
                                nc.tensor.matmul(
                                    pso_[:],
                                    oT[:, co, tb * 128 : (tb + 1) * 128],
                                    wotB[:, co - 8, :],
                                    start=False,
                                    stop=(co == 15),
                                )
                            ot = outpool.tile([128, 512], fp32, tag="ot", name="ot")
                            nc.scalar.copy(ot[:], pso_[:])
                            nc.sync.dma_start(out_r[tb, :, hs], ot[:])

    nc.compile()
    return nc


def _round_tf32(a):
    """Round fp32 array to the TF32 (fp32r) grid, round-to-nearest-even."""
    u = a.view(np.uint32).copy()
    u += 0xFFF + ((u >> 13) & 1)
    u &= np.uint32(0xFFFFE000)
    return u.view(np.float32)


def _host_prep(x, wq, wk, wv, wo, start_pos):
    x = np.asarray(x, dtype=np.float32)
    wq = np.asarray(wq, dtype=np.float32)
    wk = np.asarray(wk, dtype=np.float32)
    wv = np.asarray(wv, dtype=np.float32)
    wo = np.asarray(wo, dtype=np.float32)
    sp = int(np.asarray(start_pos))

    perm = np.concatenate([np.arange(0, 128, 2), np.arange(1, 128, 2)])

    def permute_cols(w):
        n = w.shape[1]
        return np.ascontiguousarray(
            w.reshape(H, n // 128, 128)[:, :, perm].reshape(H, n)
        )

    inv_freq = 1.0 / (ROPE_BASE ** (np.arange(0, D, 2, dtype=np.float32) / D))
    t = np.arange(sp, sp + S, dtype=np.float32)
    freqs = t[None, :] * inv_freq[:, None]  # [64, S]
    sin, cos = np.sin(freqs), np.cos(freqs)
    A = np.concatenate([sin, sin], axis=0).astype(np.float32)  # [128, S]
    Bm = np.concatenate([-cos, cos], axis=0).astype(np.float32)
    scale = np.float32(1.0 / np.sqrt(np.float32(D)))
    maps = {
        "ropeAq": np.ascontiguousarray(A * scale),
        "ropeBq": np.ascontiguousarray(Bm * scale),
        "ropeAk": A,
        "ropeBk": Bm,
    }

    in_maps = []
    for c in range(NCORES):
        b, j = divmod(c, 2)
        im = {
            "xT": _round_tf32(np.ascontiguousarray(x[b].T)),
            "wq": _round_tf32(permute_cols(wq[:, j * QC : (j + 1) * QC])),
            "wk": _round_tf32(permute_cols(wk[:, j * KC : (j + 1) * KC])),
            "wv": _round_tf32(np.ascontiguousarray(wv[:, j * VC : (j + 1) * VC])),
            "wo": _round_tf32(np.ascontiguousarray(wo[j * COH : (j + 1) * COH, :])),
        }
        im.update(maps)
        in_maps.append(im)
    return in_maps


def kernel(x, wq, wk, wv, wo, start_pos=0, _trace=False):
    from concourse.bass_utils import run_bass_kernel_spmd

    if "nc" not in _CACHE:
        _CACHE["nc"] = _build()
    nc = _CACHE["nc"]

    in_maps = _host_prep(x, wq, wk, wv, wo, start_pos)
    res = run_bass_kernel_spmd(nc, in_maps, core_ids=list(range(NCORES)), trace=_trace)
    _CACHE["last_result"] = res

    out = np.empty((B, S, H), dtype=np.float32)
    for b in range(B):
        out[b] = res.results[2 * b]["out"] + res.results[2 * b + 1]["out"]
    return out

